# revision 5
# baseline (speedup 1.0000x reference)
"""Bass/Trainium2 kernel for nn_MultiHeadAttentionBlock_23502061043960.

Reference math (note: the module multiplies RAW scores with value — no
softmax in the output path — so the whole block is linear):

    out = (concat_h Q_h (K_h^T V_h) / 8) @ w_o.T + b_o
        where Q = q w_q^T, K = k w_k^T, V = v w_v^T   (biases are zero)

Linearity lets us contract the sequence dim first:

    A_b   = k_b^T v_b                       [512, 512]   (per batch)
    M_h   = w_k[h] A_b w_v[h]^T / 8         [64, 64]     (per head)
    out_b = (blockdiag(M)^T (w_q q_b^T))^T-chain folded as
    out_b^T = w_o^T-applied:  out_b^T = w_o x_b^T,  x_b^T = BD(M)^T Qp^T,
    Qp^T = w_q q_b^T

Sharding over 8 cores: core c owns batch c//4 and sequence-quarter c%4.
Phase 1 computes the partial A (1024 rows) and folds it to the per-head
M blocks locally (fold is linear, so it commutes with the cross-core
sum); a 128 KiB AllReduce within each 4-core batch group completes M.
Phase 2 applies the folded weights to the core's own q rows.

q is staged host-side as q^T (and the output is returned as out^T)
because the PE array contracts over the partition dim; weights are
staged as W^T so they can be the stationary operand directly.
"""

import numpy as np

import concourse.bass as bass
import concourse.mybir as mybir
import concourse.tile as tile
from concourse import bacc
from concourse.bass_utils import run_bass_kernel_spmd

B = 2
S = 4096
D = 512
H = 8
DK = 64
N_CORES = 8
SQ = S // 4  # 1024 sequence rows per core
P = 128
F32 = mybir.dt.float32

# Set by tests to try the fast-fp32 TensorE mode.
USE_F32R = True

_compiled = {}

LAST_RESULTS = None  # test harness reads exec_time_ns / trace from here
RUN_KW = {}  # test harness can inject trace kwargs


def _build():
    nc = bacc.Bacc()

    DT_MM = mybir.dt.float32r if USE_F32R else F32

    kq = nc.declare_dram_parameter("kq", [SQ, D], DT_MM, isOutput=False)
    vq = nc.declare_dram_parameter("vq", [SQ, D], DT_MM, isOutput=False)
    qT = nc.declare_dram_parameter("qT", [D, SQ], DT_MM, isOutput=False)
    wkT = nc.declare_dram_parameter("wkT", [D, D], DT_MM, isOutput=False)
    wvT = nc.declare_dram_parameter("wvT", [D, D], DT_MM, isOutput=False)
    wqT = nc.declare_dram_parameter("wqT", [D, D], DT_MM, isOutput=False)
    woT = nc.declare_dram_parameter("woT", [D, D], DT_MM, isOutput=False)
    bo = nc.declare_dram_parameter("bo", [P, 4], F32, isOutput=False)
    outT = nc.declare_dram_parameter("outT", [D, SQ], F32, isOutput=True)

    kq_v = kq.rearrange("(n p) d -> n p d", p=P)  # 8 x [128, 512]
    vq_v = vq.rearrange("(n p) d -> n p d", p=P)
    qT_v = qT.rearrange("(n p) d -> n p d", p=P)  # 4 x [128, 1024]
    wkT_v = wkT.rearrange("(n p) d -> n p d", p=P)  # 4 x [128, 512]
    wvT_v = wvT.rearrange("(n p) d -> n p d", p=P)
    wqT_v = wqT.rearrange("(n p) d -> n p d", p=P)
    woT_v = woT.rearrange("(n p) d -> n p d", p=P)
    outT_v = outT.rearrange("(n p) d -> n p d", p=P)

    NKC = SQ // P  # 8 contraction chunks for A
    NDC = D // P  # 4 chunks of the model dim

    with tile.TileContext(nc) as tc:
        with (
            tc.tile_pool(name="w", bufs=1) as wp,
            tc.tile_pool(name="kv", bufs=1) as kvp,
            tc.tile_pool(name="qt", bufs=1) as qtp,
            tc.tile_pool(name="work", bufs=NDC) as wk,
            tc.tile_pool(name="big", bufs=NDC) as bigp,
            tc.tile_pool(name="small", bufs=1) as smallp,
            tc.tile_pool(name="ps", bufs=4, space="PSUM") as psp,
            tc.tile_pool(name="mps", bufs=1, space="PSUM") as mpsp,
            tc.tile_pool(name="dram", bufs=1, space="DRAM") as dramp,
        ):
            # ---- weight / bias loads -------------------------------------
            wk_t = [wp.tile([P, D], DT_MM, name=f"wk{i}", tag=f"wk{i}") for i in range(NDC)]
            wv_t = [wp.tile([P, D], DT_MM, name=f"wv{i}", tag=f"wv{i}") for i in range(NDC)]
            wq_t = [wp.tile([P, D], DT_MM, name=f"wq{i}", tag=f"wq{i}") for i in range(NDC)]
            wo_t = [wp.tile([P, D], DT_MM, name=f"wo{i}", tag=f"wo{i}") for i in range(NDC)]
            bo_t = wp.tile([P, 4], F32, name="bo", tag="bo")
            for i in range(NDC):
                nc.sync.dma_start(out=wk_t[i][:], in_=wkT_v[i])
                nc.sync.dma_start(out=wv_t[i][:], in_=wvT_v[i])
                nc.sync.dma_start(out=wq_t[i][:], in_=wqT_v[i])
                nc.sync.dma_start(out=wo_t[i][:], in_=woT_v[i])
            nc.sync.dma_start(out=bo_t[:], in_=bo[:])

            # ---- activation loads ----------------------------------------
            k_t = [kvp.tile([P, D], DT_MM, name=f"k{i}", tag=f"k{i}") for i in range(NKC)]
            v_t = [kvp.tile([P, D], DT_MM, name=f"v{i}", tag=f"v{i}") for i in range(NKC)]
            for i in range(NKC):
                nc.sync.dma_start(out=k_t[i][:], in_=kq_v[i])
                nc.sync.dma_start(out=v_t[i][:], in_=vq_v[i])
            qt_t = [qtp.tile([P, SQ], DT_MM, name=f"q{i}", tag=f"q{i}") for i in range(NDC)]
            for i in range(NDC):
                nc.sync.dma_start(out=qt_t[i][:], in_=qT_v[i])

            # ---- phase 1: A = k^T v (partial over this core's rows) ------
            a_sb = []
            for m in range(NDC):
                a_ps = psp.tile([P, D], F32, name="ps", tag="ps")
                for kc in range(NKC):
                    nc.tensor.matmul(
                        a_ps[:],
                        k_t[kc][:, m * P : (m + 1) * P],
                        v_t[kc][:],
                        start=(kc == 0),
                        stop=(kc == NKC - 1),
                    )
                t = wk.tile([P, D], DT_MM, name="a", tag="a")
                nc.vector.tensor_copy(t[:], a_ps[:])
                a_sb.append(t)

            # ---- fold F1: Y^T = A^T wkT  (Y = w_k A) ---------------------
            yT_sb = []
            for m in range(NDC):
                y_ps = psp.tile([P, D], F32, name="ps", tag="ps")
                for kc in range(NDC):
                    nc.tensor.matmul(
                        y_ps[:],
                        a_sb[kc][:, m * P : (m + 1) * P],
                        wk_t[kc][:],
                        start=(kc == 0),
                        stop=(kc == NDC - 1),
                    )
                t = wk.tile([P, D], DT_MM, name="yT", tag="yT")
                nc.vector.tensor_copy(t[:], y_ps[:])
                yT_sb.append(t)

            # ---- fold F2: M_h = (Y^T_h)^T wvT_h  = w_k[h] A w_v[h]^T -----
            m_ps = mpsp.tile([DK, D], F32, name="mps", tag="mps")
            for h in range(H):
                hs = slice(h * DK, (h + 1) * DK)
                for kc in range(NDC):
                    nc.tensor.matmul(
                        m_ps[:, hs],
                        yT_sb[kc][:, hs],
                        wv_t[kc][:, hs],
                        start=(kc == 0),
                        stop=(kc == NDC - 1),
                    )
            m_loc = smallp.tile([DK, D], F32, name="mloc", tag="mloc")
            # fold the 1/sqrt(dk)=1/8 score scale into M
            nc.scalar.mul(m_loc[:], m_ps[:], 0.125)

            # ---- AllReduce M within the 4-core batch group ---------------
            m_in = dramp.tile([DK, D], F32, name="min", tag="min")
            m_out = dramp.tile([DK, D], F32, name="mout", tag="mout")
            nc.sync.dma_start(out=m_in[:], in_=m_loc[:])
            nc.gpsimd.collective_compute(
                "AllReduce",
                mybir.AluOpType.add,
                replica_groups=[[0, 1, 2, 3], [4, 5, 6, 7]],
                ins=[m_in.opt()],
                outs=[m_out.opt()],
            )
            m_red = smallp.tile([DK, D], F32, name="mred", tag="mred")
            nc.sync.dma_start(out=m_red[:], in_=m_out[:])

            # ---- blockdiag(M) pair tiles ---------------------------------
            bd_t = []
            for j in range(4):
                t = smallp.tile([P, P], DT_MM, name=f"bd{j}", tag=f"bd{j}")
                nc.gpsimd.memset(t[:].bitcast(mybir.dt.uint32), 0)
                nc.vector.tensor_copy(
                    t[0:DK, 0:DK], m_red[:, (2 * j) * DK : (2 * j + 1) * DK]
                )
                nc.vector.tensor_copy(
                    t[DK:P, DK:P], m_red[:, (2 * j + 1) * DK : (2 * j + 2) * DK]
                )
                bd_t.append(t)

            # ---- phase 2a: Qp^T = wqT^T q^T  (= w_q q^T) -----------------
            qp_sb = [bigp.tile([P, SQ], DT_MM, name=f"qp{m}", tag="qp") for m in range(NDC)]
            for m in range(NDC):
                for nn in range(SQ // D):
                    ns = slice(nn * D, (nn + 1) * D)
                    q_ps = psp.tile([P, D], F32, name="ps", tag="ps")
                    for kc in range(NDC):
                        nc.tensor.matmul(
                            q_ps[:],
                            wq_t[kc][:, m * P : (m + 1) * P],
                            qt_t[kc][:, ns],
                            start=(kc == 0),
                            stop=(kc == NDC - 1),
                        )
                    nc.vector.tensor_copy(qp_sb[m][:, ns], q_ps[:])

            # ---- phase 2b: X^T = BD(M)^T Qp^T ----------------------------
            xT_sb = [bigp.tile([P, SQ], DT_MM, name=f"xT{j}", tag="xT") for j in range(NDC)]
            for j in range(NDC):
                for nn in range(SQ // D):
                    ns = slice(nn * D, (nn + 1) * D)
                    x_ps = psp.tile([P, D], F32, name="ps", tag="ps")
                    nc.tensor.matmul(
                        x_ps[:],
                        bd_t[j][:],
                        qp_sb[j][:, ns],
                        start=True,
                        stop=True,
                    )
                    nc.vector.tensor_copy(xT_sb[j][:, ns], x_ps[:])

            # ---- phase 2c: out^T = woT^T X^T + b_o  (= w_o x^T + b_o) ----
            for m in range(NDC):
                o_sb = bigp.tile([P, SQ], F32, name="ot", tag="ot")
                for nn in range(SQ // D):
                    ns = slice(nn * D, (nn + 1) * D)
                    o_ps = psp.tile([P, D], F32, name="ps", tag="ps")
                    for kc in range(NDC):
                        nc.tensor.matmul(
                            o_ps[:],
                            wo_t[kc][:, m * P : (m + 1) * P],
                            xT_sb[kc][:, ns],
                            start=(kc == 0),
                            stop=(kc == NDC - 1),
                        )
                    nc.scalar.activation(
                        o_sb[:, ns],
                        o_ps[:],
                        mybir.ActivationFunctionType.Identity,
                        bias=bo_t[:, m : m + 1],
                    )
                nc.sync.dma_start(out=outT_v[m], in_=o_sb[:])

    nc.compile()
    return nc


def kernel(q, k, v, w_q, b_q, w_k, b_k, w_v, b_v, w_o, b_o):
    global LAST_RESULTS
    key = ("nc", USE_F32R)
    if key not in _compiled:
        _compiled[key] = _build()
    nc = _compiled[key]

    q = np.asarray(q, dtype=np.float32)
    k = np.asarray(k, dtype=np.float32)
    v = np.asarray(v, dtype=np.float32)
    wkT = np.ascontiguousarray(np.asarray(w_k, np.float32).T)
    wvT = np.ascontiguousarray(np.asarray(w_v, np.float32).T)
    wqT = np.ascontiguousarray(np.asarray(w_q, np.float32).T)
    woT = np.ascontiguousarray(np.asarray(w_o, np.float32).T)
    bo = np.ascontiguousarray(np.asarray(b_o, np.float32).reshape(4, P).T)

    in_maps = []
    for c in range(N_CORES):
        b, quarter = divmod(c, 4)
        rows = slice(quarter * SQ, (quarter + 1) * SQ)
        in_maps.append(
            {
                "kq": np.ascontiguousarray(k[b, rows, :]),
                "vq": np.ascontiguousarray(v[b, rows, :]),
                "qT": np.ascontiguousarray(q[b, rows, :].T),
                "wkT": wkT,
                "wvT": wvT,
                "wqT": wqT,
                "woT": woT,
                "bo": bo,
            }
        )

    res = run_bass_kernel_spmd(nc, in_maps, list(range(N_CORES)), **RUN_KW)
    LAST_RESULTS = res

    out = np.empty((B, S, D), dtype=np.float32)
    for c in range(N_CORES):
        b, quarter = divmod(c, 4)
        rows = slice(quarter * SQ, (quarter + 1) * SQ)
        out[b, rows, :] = res.results[c]["outT"].T
    return out
